# revision 11
# baseline (speedup 1.0000x reference)
"""Trainium2 Bass kernel for the labelled contrastive loss.

Math (per batch row b, label L, over C=200 centers):
    cos[b,c] = <f_b, c_c> / (|f_b| |c_c|)
    a = |cos|;  l1_b = sum_c a[b,c];  row term = (2*a[b,L_b] - l1_b)/l1_b
    loss = -sum over labelled rows of row term
The feature norm |f_b| cancels in the ratio, so the kernel never computes
it: it works on raw = f @ cn^T with cn = centers/max(|c|,eps) normalized on
host (O(C*D), negligible), and forms (2*T - S)/S with
    S = sum_c |raw|,  T = |raw[b, L_b]|.

Sharding: data-parallel over the batch axis, 4096 rows per core across
8 cores; centers replicated. Per-core output is a [128,1] vector of
per-partition partial sums; the host adds them up and negates.

Device pipeline, two 128-row tiles ("a pair") at a time:
    DMA   : feature chunks [128d x 2 x 6 x 128b] (host pre-transposed so the
            contraction dim is on partitions -- no on-chip transposes)
    PE    : 2x6 accumulating matmuls (bf16 in, f32 PSUM) -> cos pair
            [128b, 2, 200c] in a single PSUM bank
    ACT   : per tile, Abs with accum_out -> exact f32 S column (the |cos|
            output itself is a throwaway; only the accumulator is used)
    DVE   : one-hot mask = is_equal(iota, label broadcast); signed
            T = rowsum(cos * mask), batched over the pair; f32 throughout
Epilogue on [128, 32] f32 tiles: T=|T|; msk * (2T - S)/S; row-reduce; DMA.

bf16 is used only for the matmul inputs; S is accumulated in f32 from the
f32 PSUM and the final ratio is f32, so input rounding enters the per-row
term only at second order (measured ~1e-7 relative on the final scalar).
"""

import numpy as np
import ml_dtypes

import concourse.bass as bass
import concourse.tile as tile
from concourse import mybir
from concourse.bass_utils import run_bass_kernel_spmd

# ---------------------------------------------------------------------------
# Workaround for walrus "Too many sync wait commands": this toolchain only
# encodes a limited number of sem waits per instruction, so spread excess
# waits over preceding same-engine nops — both for scheduled instructions
# (pre-lowering pass) and for the TileContext tail drain.
# ---------------------------------------------------------------------------
from concourse.vector_clock import ScopedClock

_MAX_WAITS = 1
_split_counter = [0]


def _split_waits_in_ordered(ordered):
    for bb_name, insts in ordered.items():
        new = []
        for inst in insts:
            si = getattr(inst, "sync_info", None)
            waits = list(si.on_wait) if si is not None and si.on_wait else []
            if len(waits) > _MAX_WAITS:
                updates = list(si.on_update) if si.on_update else []
                head, tail = waits[:-_MAX_WAITS], waits[-_MAX_WAITS:]
                while head:
                    n = mybir.InstNoOp(
                        name=f"I-wsplit-{_split_counter[0]}", ins=[], outs=[]
                    )
                    _split_counter[0] += 1
                    n.engine = inst.engine
                    n.bass_nofuse = True
                    n.sync_info = mybir.SyncInfo(
                        on_wait=head[:_MAX_WAITS], on_update=[]
                    )
                    head = head[_MAX_WAITS:]
                    new.append(n)
                inst.sync_info = mybir.SyncInfo(on_wait=tail, on_update=updates)
            new.append(inst)
        ordered[bb_name] = new


_orig_lower_ordered = tile.TileContext._lower_ordered_insts


def _patched_lower_ordered(self, ordered):
    _split_waits_in_ordered(ordered)
    return _orig_lower_ordered(self, ordered)


tile.TileContext._lower_ordered_insts = _patched_lower_ordered


def _patched_drain_and_barrier(self, tick_clock, wait_clock):
    """Minimal kernel tail replacing the stock drain + two EVSEM-butterfly
    barriers (~15us):

    1. SP nops carry one sem wait each for every proc's final clock tick —
       once they pass, every tracked semaphore increment has LANDED (waits
       observe the final value of each proc's latest sem; same-engine and
       same-queue increments retire in order).
    2. Each engine drains its pipeline and bumps a tail semaphore; once it
       passes its own last wait nothing can block it, so this retires.
    3. GpSimd waits for the 4 other engines + SP, then range-clears all
       tile semaphores, resets DMA queue state and clears the tail sem.
    4. Engines halt independently; the NEFF only completes (and can only
       be re-executed) when every engine including GpSimd has halted, so
       the next run starts with everything zeroed.
    """
    nc = self.nc
    carrier = nc.sync.nop(nofuse=True)
    wait_clock.add_sem_waits(carrier.ins, ScopedClock({None: tick_clock.global_clock}))
    si = carrier.ins.sync_info
    waits = list(si.on_wait) if si is not None and si.on_wait else []
    if len(waits) > _MAX_WAITS:
        updates = list(si.on_update) if si.on_update else []
        carrier.ins.sync_info = mybir.SyncInfo(on_wait=[], on_update=updates)
        rest = waits
        while rest:
            n = nc.sync.nop(nofuse=True)
            n.ins.sync_info = mybir.SyncInfo(on_wait=rest[:_MAX_WAITS], on_update=[])
            rest = rest[_MAX_WAITS:]
    nc.sync.drain()

    tail_sem = nc.alloc_semaphore("tile_tail_sem")
    n_inc = 0
    for eng_type, eng in nc.engines.items():
        if eng_type == mybir.EngineType.Pool:
            continue
        eng.drain()
        eng.sem_inc(tail_sem, 1)
        n_inc += 1
    nc.gpsimd.drain()
    nc.gpsimd.wait_ge(tail_sem, n_inc)

    assert self.sems is not None
    popped = nc._tile_sem_poison_stack.pop()
    assert popped is self._sem_poison
    nc.clear_and_free_semaphores(list(self.sems.allocated().values()))
    nc.clear_and_free_semaphores([tail_sem])


tile.TileContext._drain_and_barrier = _patched_drain_and_barrier

# ---------------------------------------------------------------------------
# Problem constants (hardcoded per contract)
# ---------------------------------------------------------------------------
N_CORES = 8
B, D, C = 32768, 768, 200
B_CORE = B // N_CORES          # 4096
P = 128                        # partitions
KCH = D // P                   # 6 contraction chunks
NT = B_CORE // P               # 32 tiles per core
NPAIR = NT // 2                # 16 pairs
EPS_COS = 1e-8

_TRACE = False                 # test.py flips this for profiling runs
_TRACE_DIR = None
last_results = None

_nc = None


def _build():
    global _nc
    if _nc is not None:
        return _nc
    nc = bass.Bass("TRN2", debug=False, num_devices=N_CORES)

    bf16 = mybir.dt.bfloat16
    f32 = mybir.dt.float32

    # ft[pair, p, t', k, b] = features[(2*pair+t')*128 + b, k*128 + p], bf16
    ft = nc.dram_tensor("ft", [NPAIR, P, 2, KCH, P], bf16, kind="ExternalInput")
    cnt = nc.dram_tensor("cnt", [P, KCH, C], bf16, kind="ExternalInput")
    iota = nc.dram_tensor("iota", [P, 4, C], f32, kind="ExternalInput")
    lab = nc.dram_tensor("lab", [P, NT], f32, kind="ExternalInput")
    msk = nc.dram_tensor("msk", [P, NT], f32, kind="ExternalInput")
    out = nc.dram_tensor("out", [P, 1], f32, kind="ExternalOutput")

    with tile.TileContext(nc) as tc:
        with (
            tc.tile_pool(name="singles", bufs=1) as singles,
            tc.tile_pool(name="ftp", bufs=6) as ftp,
            tc.tile_pool(name="work", bufs=4) as work,
            tc.tile_pool(name="psum", bufs=4, space="PSUM") as psum,
        ):
            cnt_sb = singles.tile([P, KCH, C], bf16)
            nc.sync.dma_start(cnt_sb[:], cnt[:])
            iota_sb = singles.tile([P, 4, C], f32)
            nc.sync.dma_start(iota_sb[:], iota[:])
            lab_sb = singles.tile([P, NT], f32)
            nc.sync.dma_start(lab_sb[:], lab[:])
            msk_sb = singles.tile([P, NT], f32)
            nc.sync.dma_start(msk_sb[:], msk[:])

            s_all = singles.tile([P, NT], f32)
            t_all = singles.tile([P, NT], f32)

            for pr in range(NPAIR):
                t0 = 2 * pr
                ft_sb = ftp.tile([P, 2, KCH, P], bf16)
                nc.sync.dma_start(ft_sb[:], ft[pr])

                # one-hot masks for 4 tiles at a time (2 pairs)
                if pr % 2 == 0:
                    mask_sb = work.tile([P, 4, C], f32, tag="mask")
                    nc.vector.tensor_tensor(
                        out=mask_sb[:],
                        in0=iota_sb[:],
                        in1=lab_sb[:, t0 : t0 + 4].broadcast_to([P, 4, C]),
                        op=mybir.AluOpType.is_equal,
                    )
                mhalf = (pr % 2) * 2

                cos_ps = psum.tile([P, 2, C], f32)
                for j in range(2):
                    for k in range(KCH):
                        nc.tensor.matmul(
                            cos_ps[:, j, :],
                            ft_sb[:, j, k, :],
                            cnt_sb[:, k, :],
                            start=(k == 0),
                            stop=(k == KCH - 1),
                        )

                # S columns: ACT Abs with row-sum accumulator (out is junk)
                junk_sb = work.tile([P, 2, C], bf16, tag="junk")
                for j in range(2):
                    nc.scalar.activation(
                        out=junk_sb[:, j, :],
                        in_=cos_ps[:, j, :],
                        func=mybir.ActivationFunctionType.Abs,
                        accum_out=s_all[:, t0 + j : t0 + j + 1],
                    )

                # signed T columns for the pair on DVE (f32)
                am_sb = work.tile([P, 2, C], f32, tag="am")
                nc.vector.tensor_tensor(
                    out=am_sb[:], in0=cos_ps[:],
                    in1=mask_sb[:, mhalf : mhalf + 2, :],
                    op=mybir.AluOpType.mult,
                )
                nc.vector.tensor_reduce(
                    out=t_all[:, t0 : t0 + 2], in_=am_sb[:],
                    op=mybir.AluOpType.add, axis=mybir.AxisListType.X,
                )

            # epilogue: T = |T|; per-row term = msk * (2*T - S) / S; reduce
            t_abs = singles.tile([P, NT], f32)
            nc.scalar.activation(
                out=t_abs[:], in_=t_all[:],
                func=mybir.ActivationFunctionType.Abs,
            )
            recip = singles.tile([P, NT], f32)
            nc.vector.reciprocal(recip[:], s_all[:])
            num = singles.tile([P, NT], f32)
            nc.vector.tensor_scalar(
                out=num[:],
                in0=t_abs[:],
                scalar1=2.0,
                scalar2=None,
                op0=mybir.AluOpType.mult,
            )
            nc.vector.tensor_tensor(
                out=num[:], in0=num[:], in1=s_all[:], op=mybir.AluOpType.subtract
            )
            nc.vector.tensor_tensor(
                out=num[:], in0=num[:], in1=recip[:], op=mybir.AluOpType.mult
            )
            nc.vector.tensor_tensor(
                out=num[:], in0=num[:], in1=msk_sb[:], op=mybir.AluOpType.mult
            )
            out_col = singles.tile([P, 1], f32)
            nc.vector.tensor_reduce(
                out=out_col[:], in_=num[:], op=mybir.AluOpType.add,
                axis=mybir.AxisListType.X,
            )
            nc.sync.dma_start(out[:], out_col[:])

    _nc = nc
    return nc


def kernel(features, centers, labels, labelled_or_not):
    global last_results
    nc = _build()

    bf = ml_dtypes.bfloat16
    features = np.asarray(features, dtype=np.float32)
    centers = np.asarray(centers, dtype=np.float32)
    labels_f = np.asarray(labels).astype(np.float32)
    msk_f = np.asarray(labelled_or_not).astype(np.float32)

    # normalized + transposed centers -> [P, KCH, C] in bf16
    cn = centers / np.maximum(
        np.linalg.norm(centers, axis=1, keepdims=True), EPS_COS
    )
    cnt_host = np.ascontiguousarray(
        cn.reshape(C, KCH, P).transpose(2, 1, 0).astype(bf)
    )
    iota_host = np.ascontiguousarray(
        np.broadcast_to(np.arange(C, dtype=np.float32), (P, 4, C))
    )

    in_maps = []
    for c in range(N_CORES):
        sl = slice(c * B_CORE, (c + 1) * B_CORE)
        fcore = features[sl]  # [4096, 768]
        # ft[pair, p, t', k, b] = f[(2*pair+t')*128 + b, k*128 + p]
        ft_host = np.ascontiguousarray(
            fcore.reshape(NPAIR, 2, P, KCH, P).transpose(0, 4, 1, 3, 2).astype(bf)
        )
        lab_host = np.ascontiguousarray(labels_f[sl].reshape(NT, P).T)
        msk_host = np.ascontiguousarray(msk_f[sl].reshape(NT, P).T)
        in_maps.append(
            {
                "ft": ft_host,
                "cnt": cnt_host,
                "iota": iota_host,
                "lab": lab_host,
                "msk": msk_host,
            }
        )

    kwargs = {}
    if _TRACE:
        kwargs["trace"] = True
        if _TRACE_DIR:
            kwargs["tmpdir"] = _TRACE_DIR
    res = run_bass_kernel_spmd(nc, in_maps, core_ids=list(range(N_CORES)), **kwargs)
    last_results = res

    total = 0.0
    for c in range(N_CORES):
        total += res.results[c]["out"].astype(np.float64).sum()
    return np.array(-total, dtype=np.float32)


# revision 14
# speedup vs baseline: 1.1787x; 1.1787x over previous
"""Trainium2 Bass kernel for the labelled contrastive loss.

Math (per batch row b, label L, over C=200 centers):
    cos[b,c] = <f_b, c_c> / (|f_b| |c_c|)
    a = |cos|;  l1_b = sum_c a[b,c];  row term = (2*a[b,L_b] - l1_b)/l1_b
    loss = -sum over labelled rows of row term
The feature norm |f_b| cancels in the ratio, so the kernel never computes
it: it works on raw = f @ cn^T with cn = centers/max(|c|,eps) normalized on
host (O(C*D), negligible), and forms (2*T - S)/S with
    S = sum_c |raw|,  T = |raw[b, L_b]|.

Sharding: data-parallel over the batch axis, 4096 rows per core across
8 cores; centers replicated. Per-core output is a [128,1] vector of
per-partition partial sums; the host adds them up and negates.

Device pipeline, two 128-row tiles ("a pair") at a time:
    DMA   : feature chunks [128d x 2 x 6 x 128b] (host pre-transposed so the
            contraction dim is on partitions -- no on-chip transposes)
    PE    : 2x6 accumulating matmuls (bf16 in, f32 PSUM) -> cos pair
            [128b, 2, 200c] in a single PSUM bank
    ACT   : per tile, Abs with accum_out -> exact f32 S column (the |cos|
            output itself is a throwaway; only the accumulator is used)
    DVE   : one-hot mask = is_equal(iota, label broadcast); signed
            T = rowsum(cos * mask), batched over the pair; f32 throughout
Epilogue on [128, 32] f32 tiles: T=|T|; msk * (2T - S)/S; row-reduce; DMA.

bf16 is used only for the matmul inputs; S is accumulated in f32 from the
f32 PSUM and the final ratio is f32, so input rounding enters the per-row
term only at second order (measured ~1e-7 relative on the final scalar).
"""

import numpy as np
import ml_dtypes

import concourse.bass as bass
import concourse.tile as tile
from concourse import mybir
from concourse.bass_utils import run_bass_kernel_spmd

# ---------------------------------------------------------------------------
# Workaround for walrus "Too many sync wait commands": this toolchain only
# encodes a limited number of sem waits per instruction, so spread excess
# waits over preceding same-engine nops — both for scheduled instructions
# (pre-lowering pass) and for the TileContext tail drain.
# ---------------------------------------------------------------------------
from concourse.vector_clock import ScopedClock

_MAX_WAITS = 1
_split_counter = [0]


def _split_waits_in_ordered(ordered):
    for bb_name, insts in ordered.items():
        new = []
        for inst in insts:
            si = getattr(inst, "sync_info", None)
            waits = list(si.on_wait) if si is not None and si.on_wait else []
            if len(waits) > _MAX_WAITS:
                updates = list(si.on_update) if si.on_update else []
                head, tail = waits[:-_MAX_WAITS], waits[-_MAX_WAITS:]
                while head:
                    n = mybir.InstNoOp(
                        name=f"I-wsplit-{_split_counter[0]}", ins=[], outs=[]
                    )
                    _split_counter[0] += 1
                    n.engine = inst.engine
                    n.bass_nofuse = True
                    n.sync_info = mybir.SyncInfo(
                        on_wait=head[:_MAX_WAITS], on_update=[]
                    )
                    head = head[_MAX_WAITS:]
                    new.append(n)
                inst.sync_info = mybir.SyncInfo(on_wait=tail, on_update=updates)
            new.append(inst)
        ordered[bb_name] = new


_orig_lower_ordered = tile.TileContext._lower_ordered_insts


def _patched_lower_ordered(self, ordered):
    _split_waits_in_ordered(ordered)
    return _orig_lower_ordered(self, ordered)


tile.TileContext._lower_ordered_insts = _patched_lower_ordered


def _patched_drain_and_barrier(self, tick_clock, wait_clock):
    """Minimal kernel tail replacing the stock drain + two EVSEM-butterfly
    barriers (~15us):

    1. SP nops carry one sem wait each for every proc's final clock tick —
       once they pass, every tracked semaphore increment has LANDED (waits
       observe the final value of each proc's latest sem; same-engine and
       same-queue increments retire in order).
    2. Each engine drains its pipeline and bumps a tail semaphore; once it
       passes its own last wait nothing can block it, so this retires.
    3. GpSimd waits for the 4 other engines + SP, then range-clears all
       tile semaphores, resets DMA queue state and clears the tail sem.
    4. Engines halt independently; the NEFF only completes (and can only
       be re-executed) when every engine including GpSimd has halted, so
       the next run starts with everything zeroed.
    """
    nc = self.nc
    carrier = nc.sync.nop(nofuse=True)
    wait_clock.add_sem_waits(carrier.ins, ScopedClock({None: tick_clock.global_clock}))
    si = carrier.ins.sync_info
    waits = list(si.on_wait) if si is not None and si.on_wait else []
    if len(waits) > _MAX_WAITS:
        updates = list(si.on_update) if si.on_update else []
        carrier.ins.sync_info = mybir.SyncInfo(on_wait=[], on_update=updates)
        rest = waits
        while rest:
            n = nc.sync.nop(nofuse=True)
            n.ins.sync_info = mybir.SyncInfo(on_wait=rest[:_MAX_WAITS], on_update=[])
            rest = rest[_MAX_WAITS:]
    nc.sync.drain()

    tail_sem = nc.alloc_semaphore("tile_tail_sem")
    n_inc = 0
    for eng_type, eng in nc.engines.items():
        if eng_type == mybir.EngineType.Pool:
            continue
        eng.drain()
        eng.sem_inc(tail_sem, 1)
        n_inc += 1
    nc.gpsimd.drain()
    nc.gpsimd.wait_ge(tail_sem, n_inc)

    assert self.sems is not None
    popped = nc._tile_sem_poison_stack.pop()
    assert popped is self._sem_poison
    nc.clear_and_free_semaphores(list(self.sems.allocated().values()))
    nc.clear_and_free_semaphores([tail_sem])


tile.TileContext._drain_and_barrier = _patched_drain_and_barrier

# ---------------------------------------------------------------------------
# Problem constants (hardcoded per contract)
# ---------------------------------------------------------------------------
N_CORES = 8
B, D, C = 32768, 768, 200
B_CORE = B // N_CORES          # 4096
P = 128                        # partitions
KCH = D // P                   # 6 contraction chunks
NT = B_CORE // P               # 32 tiles per core
NPAIR = NT // 2                # 16 pairs
EPS_COS = 1e-8

_TRACE = False                 # test.py flips this for profiling runs
_TRACE_DIR = None
last_results = None

_nc = None


def _build():
    global _nc
    if _nc is not None:
        return _nc
    nc = bass.Bass("TRN2", debug=False, num_devices=N_CORES)

    bf16 = mybir.dt.bfloat16
    f32 = mybir.dt.float32

    # ft[pair, p, t', k, b] = features[(2*pair+t')*128 + b, k*128 + p], bf16
    ft = nc.dram_tensor("ft", [NPAIR, P, 2, KCH, P], bf16, kind="ExternalInput")
    cnt = nc.dram_tensor("cnt", [P, KCH, C], bf16, kind="ExternalInput")
    iota = nc.dram_tensor("iota", [P, 4, C], f32, kind="ExternalInput")
    lab = nc.dram_tensor("lab", [P, NT], f32, kind="ExternalInput")
    msk = nc.dram_tensor("msk", [P, NT], f32, kind="ExternalInput")
    out = nc.dram_tensor("out", [1, 1], f32, kind="ExternalOutput")

    with tile.TileContext(nc) as tc:
        with (
            tc.tile_pool(name="singles", bufs=1) as singles,
            tc.tile_pool(name="ftp", bufs=6) as ftp,
            tc.tile_pool(name="work", bufs=4) as work,
            tc.tile_pool(name="psum", bufs=4, space="PSUM") as psum,
        ):
            cnt_sb = singles.tile([P, KCH, C], bf16)
            nc.sync.dma_start(cnt_sb[:], cnt[:])
            iota_sb = singles.tile([P, 4, C], f32)
            nc.sync.dma_start(iota_sb[:], iota[:])
            lab_sb = singles.tile([P, NT], f32)
            nc.sync.dma_start(lab_sb[:], lab[:])
            msk_sb = singles.tile([P, NT], f32)
            nc.sync.dma_start(msk_sb[:], msk[:])

            s_all = singles.tile([P, NT], f32)
            t_all = singles.tile([P, NT], f32)

            for pr in range(NPAIR):
                t0 = 2 * pr
                ft_sb = ftp.tile([P, 2, KCH, P], bf16)
                nc.sync.dma_start(ft_sb[:], ft[pr])

                # one-hot masks for 4 tiles at a time (2 pairs)
                if pr % 2 == 0:
                    mask_sb = work.tile([P, 4, C], f32, tag="mask")
                    nc.vector.tensor_tensor(
                        out=mask_sb[:],
                        in0=iota_sb[:],
                        in1=lab_sb[:, t0 : t0 + 4].broadcast_to([P, 4, C]),
                        op=mybir.AluOpType.is_equal,
                    )
                mhalf = (pr % 2) * 2

                cos_ps = psum.tile([P, 2, C], f32)
                for j in range(2):
                    for k in range(KCH):
                        nc.tensor.matmul(
                            cos_ps[:, j, :],
                            ft_sb[:, j, k, :],
                            cnt_sb[:, k, :],
                            start=(k == 0),
                            stop=(k == KCH - 1),
                        )

                # S columns: ACT Abs with row-sum accumulator (out is junk)
                junk_sb = work.tile([P, 2, C], bf16, tag="junk")
                for j in range(2):
                    nc.scalar.activation(
                        out=junk_sb[:, j, :],
                        in_=cos_ps[:, j, :],
                        func=mybir.ActivationFunctionType.Abs,
                        accum_out=s_all[:, t0 + j : t0 + j + 1],
                    )

                # signed T columns for the pair on DVE (f32)
                am_sb = work.tile([P, 2, C], f32, tag="am")
                nc.vector.tensor_tensor(
                    out=am_sb[:], in0=cos_ps[:],
                    in1=mask_sb[:, mhalf : mhalf + 2, :],
                    op=mybir.AluOpType.mult,
                )
                nc.vector.tensor_reduce(
                    out=t_all[:, t0 : t0 + 2], in_=am_sb[:],
                    op=mybir.AluOpType.add, axis=mybir.AxisListType.X,
                )

            # epilogue: T = |T|; per-row term = msk * (2*T - S) / S; reduce
            t_abs = singles.tile([P, NT], f32)
            nc.scalar.activation(
                out=t_abs[:], in_=t_all[:],
                func=mybir.ActivationFunctionType.Abs,
            )
            recip = singles.tile([P, NT], f32)
            nc.vector.reciprocal(recip[:], s_all[:])
            num = singles.tile([P, NT], f32)
            nc.vector.tensor_scalar(
                out=num[:],
                in0=t_abs[:],
                scalar1=2.0,
                scalar2=None,
                op0=mybir.AluOpType.mult,
            )
            nc.vector.tensor_tensor(
                out=num[:], in0=num[:], in1=s_all[:], op=mybir.AluOpType.subtract
            )
            nc.vector.tensor_tensor(
                out=num[:], in0=num[:], in1=recip[:], op=mybir.AluOpType.mult
            )
            nc.vector.tensor_tensor(
                out=num[:], in0=num[:], in1=msk_sb[:], op=mybir.AluOpType.mult
            )
            # collapse to one scalar on-chip: PE sums over partitions, DVE
            # over the NT columns -- so the store is a single 4B descriptor
            # (a [128,1] store would spray 128 tiny descriptors over all 16
            # DMA engines, whose completion events straggle for ~6us).
            ones_sb = singles.tile([P, 1], f32)
            nc.vector.memset(ones_sb[:], 1.0)
            tot_ps = psum.tile([1, NT], f32)
            nc.tensor.matmul(tot_ps[:], ones_sb[:], num[:], start=True, stop=True)
            out_sb = singles.tile([1, 1], f32)
            nc.vector.tensor_reduce(
                out=out_sb[:], in_=tot_ps[:], op=mybir.AluOpType.add,
                axis=mybir.AxisListType.X,
            )
            nc.sync.dma_start(out[:], out_sb[:])

    _nc = nc
    return nc


def kernel(features, centers, labels, labelled_or_not):
    global last_results
    nc = _build()

    bf = ml_dtypes.bfloat16
    features = np.asarray(features, dtype=np.float32)
    centers = np.asarray(centers, dtype=np.float32)
    labels_f = np.asarray(labels).astype(np.float32)
    msk_f = np.asarray(labelled_or_not).astype(np.float32)

    # normalized + transposed centers -> [P, KCH, C] in bf16
    cn = centers / np.maximum(
        np.linalg.norm(centers, axis=1, keepdims=True), EPS_COS
    )
    cnt_host = np.ascontiguousarray(
        cn.reshape(C, KCH, P).transpose(2, 1, 0).astype(bf)
    )
    iota_host = np.ascontiguousarray(
        np.broadcast_to(np.arange(C, dtype=np.float32), (P, 4, C))
    )

    in_maps = []
    for c in range(N_CORES):
        sl = slice(c * B_CORE, (c + 1) * B_CORE)
        fcore = features[sl]  # [4096, 768]
        # ft[pair, p, t', k, b] = f[(2*pair+t')*128 + b, k*128 + p]
        ft_host = np.ascontiguousarray(
            fcore.reshape(NPAIR, 2, P, KCH, P).transpose(0, 4, 1, 3, 2).astype(bf)
        )
        lab_host = np.ascontiguousarray(labels_f[sl].reshape(NT, P).T)
        msk_host = np.ascontiguousarray(msk_f[sl].reshape(NT, P).T)
        in_maps.append(
            {
                "ft": ft_host,
                "cnt": cnt_host,
                "iota": iota_host,
                "lab": lab_host,
                "msk": msk_host,
            }
        )

    kwargs = {}
    if _TRACE:
        kwargs["trace"] = True
        if _TRACE_DIR:
            kwargs["tmpdir"] = _TRACE_DIR
    res = run_bass_kernel_spmd(nc, in_maps, core_ids=list(range(N_CORES)), **kwargs)
    last_results = res

    total = 0.0
    for c in range(N_CORES):
        total += float(res.results[c]["out"][0, 0])
    return np.array(-total, dtype=np.float32)
